# revision 5
# baseline (speedup 1.0000x reference)
"""GCN+GIN graph encoder on 8 Trainium2 NeuronCores (Bass/Tile).

Math (reference):
  GCNConv:  h = relu(segsum_dst(norm_e * (x@W0)[src]) + b0),
            norm_e = dinv[src]*dinv[dst] over edges+self-loops,
            dinv = rsqrt(deg incl self-loop)
  GIN x2:   h = relu((h + segsum_dst(h[src])) @ Wg + bg)
  pool:     m = segment_mean(h, batch) -> relu(m@Wh1+bh1)@Wh2+bh2

Distribution: nodes (and their in-edges) sharded contiguously over 8 cores.
Per layer each core aggregates messages for its own dst nodes by gathering
rows of a replicated node-feature table (dma_gather, 1024-row packed ops on
4 SWDGE queues), reducing edge tiles with one-hot selection matrices on the
TensorEngine, applying the layer linear transform W-stationary in feat-major,
then transposing back to node-major.  Tables are re-replicated between layers
with an AllGather; pooled partial means are combined with an AllReduce and
the small MLP head is computed redundantly on every core.

Aggregation identity per dst block b (128 dst nodes):
  aggT[f, d] = sum_e msg[e, f] * sel[e, d],  sel[e, d] = (doff[e] == d) * val[e]
computed as matmul(lhsT=msg_tile[128e, 128f], rhs=sel[128e, 128d]) accumulated
in PSUM over the block's edge tiles.  GCN folds dinv[src] into the table rows
(host-prescaled x) and dinv[dst] into val; GIN uses val=1 and a self-loop edge
supplies the "+h" term.  Pad edge slots carry doff=-1 -> zero contribution.
"""
import sys
import os

sys.path.insert(0, '/opt/trn_rl_repo')

import numpy as np

import concourse.bass as bass
import concourse.bacc as bacc
import concourse.mybir as mybir
import concourse.tile as tile
from concourse.bass_utils import run_bass_kernel_spmd
from concourse.masks import make_identity

F32 = mybir.dt.float32
I16 = mybir.dt.int16
P = 128
NCORES = 8
GATHER_ROWS = 1024          # rows per dma_gather (single_packet limit)
NQ = 4                      # SWDGE queues


class Cfg:
    def __init__(self, N, E, G, F, NHID, NOUT, NPN):
        self.N = N            # real nodes
        self.E = E            # edges (no self loops)
        self.G = G            # graphs
        self.F = F            # feature/hidden width (128)
        self.NHID = NHID
        self.NOUT = NOUT
        self.NPN = NPN        # real nodes per core
        assert NPN * NCORES >= N > NPN * (NCORES - 1)
        self.NPC = ((NPN + P - 1) // P) * P   # padded nodes per core
        self.NBLK = self.NPC // P
        self.NPAD = self.NPC * NCORES
        self.NHALF = self.NPAD // 2
        assert self.NHALF < 32768
        assert G == 2 * P


FULL = Cfg(N=50000, E=800000, G=256, F=128, NHID=256, NOUT=128, NPN=6250)


# ---------------------------------------------------------------- host prep
def preprocess(cfg, x, edge_index, batch, W0, b0, Wg1, bg1, Wg2, bg2,
               Wh1, bh1, Wh2, bh2):
    N, G, F = cfg.N, cfg.G, cfg.F
    NPN, NPC, NBLK, NPAD, NHALF = cfg.NPN, cfg.NPC, cfg.NBLK, cfg.NPAD, cfg.NHALF

    src = np.asarray(edge_index[0], dtype=np.int64)
    dst = np.asarray(edge_index[1], dtype=np.int64)
    batch = np.asarray(batch, dtype=np.int64)
    loop = np.arange(N, dtype=np.int64)
    s_all = np.concatenate([src, loop])
    d_all = np.concatenate([dst, loop])

    deg = np.bincount(d_all, minlength=N).astype(np.float64)
    dinv = (1.0 / np.sqrt(np.maximum(deg, 1.0))).astype(np.float32)

    def tabidx(n):
        c = n // NPN
        return c * NPC + (n - c * NPN)

    sidx = tabidx(s_all).astype(np.int64)
    c_e = d_all // NPN
    loc = d_all - c_e * NPN
    b_e = loc // P
    off_e = loc % P
    gblk = c_e * NBLK + b_e                      # global dst block id
    val_e = dinv[d_all].astype(np.float32)      # GCN dst scaling

    NGB = NCORES * NBLK
    streams = {}
    for name, mask in (("lo", sidx < NHALF), ("hi", sidx >= NHALF)):
        sg = gblk[mask]
        si = sidx[mask] - (0 if name == "lo" else NHALF)
        sof = off_e[mask]
        sva = val_e[mask]
        order = np.argsort(sg, kind="stable")
        sg, si, sof, sva = sg[order], si[order], sof[order], sva[order]
        cnt = np.bincount(sg, minlength=NGB)
        NT = int(np.ceil(cnt.max() / P)) if len(sg) else 1
        NT = max(NT, 1)
        rows_blk = NT * P
        starts = np.zeros(NGB, dtype=np.int64)
        starts[1:] = np.cumsum(cnt)[:-1]
        rank = np.arange(len(sg)) - np.repeat(starts, cnt)
        pos = sg * rows_blk + rank               # row in padded stream
        tot = NGB * rows_blk
        idx_arr = np.zeros(tot, dtype=np.int32)
        doff_arr = np.full(tot, -1.0, dtype=np.float32)
        val_arr = np.zeros(tot, dtype=np.float32)
        idx_arr[pos] = si
        doff_arr[pos] = sof
        val_arr[pos] = sva
        # per-core views
        rows_core = NBLK * rows_blk
        idx_arr = idx_arr.reshape(NCORES, rows_core)
        doff_arr = doff_arr.reshape(NCORES, rows_core)
        val_arr = val_arr.reshape(NCORES, rows_core)
        NG = (rows_core + GATHER_ROWS - 1) // GATHER_ROWS
        rows_g = NG * GATHER_ROWS
        pad = rows_g - rows_core
        if pad:
            idx_arr = np.pad(idx_arr, ((0, 0), (0, pad)))
        # wrap int16 for dma_gather: element i -> partition i%16, col i//16
        NWG = GATHER_ROWS // 16
        wrapped = idx_arr.reshape(NCORES, NG, NWG, 16).transpose(0, 3, 1, 2)
        wrapped = wrapped.reshape(NCORES, 16, NG * NWG).astype(np.int16)
        wrapped = np.tile(wrapped, (1, 8, 1))    # [NCORES, 128, NG*NWG]
        # doff/val tile-major: [T=NBLK*NT, 128] -> [128, T]
        T = NBLK * NT
        doff2 = doff_arr.reshape(NCORES, T, P).transpose(0, 2, 1).copy()
        val2 = val_arr.reshape(NCORES, T, P).transpose(0, 2, 1).copy()
        streams[name] = dict(NT=NT, NG=NG, idx=wrapped, doff=doff2, val=val2)

    # per-core node-feature slice, pre-scaled by dinv (GCN source scaling)
    xs = np.zeros((NCORES, NPC, F), dtype=np.float32)
    x = np.asarray(x, dtype=np.float32)
    for c in range(NCORES):
        lo_n = c * NPN
        hi_n = min(N, (c + 1) * NPN)
        n = hi_n - lo_n
        xs[c, :n] = x[lo_n:hi_n] * dinv[lo_n:hi_n, None]

    # pooling metadata
    cnt_g = np.bincount(batch, minlength=G).astype(np.float32)
    invc = (1.0 / np.maximum(cnt_g, 1.0)).astype(np.float32)
    batA = np.full((NCORES, P, NBLK), -1.0, dtype=np.float32)
    batB = np.full((NCORES, P, NBLK), -1000.0, dtype=np.float32)
    for c in range(NCORES):
        lo_n = c * NPN
        hi_n = min(N, (c + 1) * NPN)
        n = hi_n - lo_n
        bb = batch[lo_n:hi_n].astype(np.float32)
        colmaj = np.full(NPC, -1.0, dtype=np.float32)
        colmaj[:n] = bb
        batA[c] = colmaj.reshape(NBLK, P).T
        batB[c] = batA[c] - 128.0
        batA[c][batA[c] < 0] = -1.0

    iota = np.broadcast_to(np.arange(P, dtype=np.float32), (P, P)).copy()

    common = dict(
        iota=iota,
        w0=np.asarray(W0, np.float32), wg1=np.asarray(Wg1, np.float32),
        wg2=np.asarray(Wg2, np.float32),
        b0c=np.asarray(b0, np.float32).reshape(P, 1).copy(),
        bg1c=np.asarray(bg1, np.float32).reshape(P, 1).copy(),
        bg2c=np.asarray(bg2, np.float32).reshape(P, 1).copy(),
        wh1=np.asarray(Wh1, np.float32),
        bh1c=np.asarray(bh1, np.float32).reshape(2, P).T.copy(),  # [128,2]
        wh2=np.asarray(Wh2, np.float32),
        bh2rep=np.broadcast_to(np.asarray(bh2, np.float32), (P, cfg.NOUT)).copy(),
        invcA=invc[:P].reshape(P, 1).copy(),
        invcB=invc[P:].reshape(P, 1).copy(),
    )
    in_maps = []
    for c in range(NCORES):
        m = dict(common)
        m.update(
            xs=xs[c],
            idxlo=streams["lo"]["idx"][c], idxhi=streams["hi"]["idx"][c],
            dofflo=streams["lo"]["doff"][c], doffhi=streams["hi"]["doff"][c],
            vallo=streams["lo"]["val"][c], valhi=streams["hi"]["val"][c],
            batA=batA[c], batB=batB[c],
        )
        in_maps.append(m)
    meta = dict(NTLO=streams["lo"]["NT"], NGLO=streams["lo"]["NG"],
                NTHI=streams["hi"]["NT"], NGHI=streams["hi"]["NG"])
    return in_maps, meta


# ---------------------------------------------------------------- program
def build_program(cfg, meta):
    NPC, NBLK, NPAD, NHALF = cfg.NPC, cfg.NBLK, cfg.NPAD, cfg.NHALF
    F, NHID, NOUT, G = cfg.F, cfg.NHID, cfg.NOUT, cfg.G
    NTLO, NGLO = meta["NTLO"], meta["NGLO"]
    NTHI, NGHI = meta["NTHI"], meta["NGHI"]
    NWG = GATHER_ROWS // 16
    CHUNKS = GATHER_ROWS // P     # 8 message tiles per gather

    nc = bacc.Bacc(None, target_bir_lowering=False, debug=True,
                   num_devices=NCORES, num_swdge_queues=NQ)

    def din(name, shape, dt=F32):
        return nc.declare_dram_parameter(name, list(shape), dt, isOutput=False)

    xs_d = din("xs", [NPC, F])
    idxlo_d = din("idxlo", [P, NGLO * NWG], I16)
    idxhi_d = din("idxhi", [P, NGHI * NWG], I16)
    dofflo_d = din("dofflo", [P, NBLK * NTLO])
    doffhi_d = din("doffhi", [P, NBLK * NTHI])
    vallo_d = din("vallo", [P, NBLK * NTLO])
    valhi_d = din("valhi", [P, NBLK * NTHI])
    iota_d = din("iota", [P, P])
    w0_d = din("w0", [F, F]); wg1_d = din("wg1", [F, F]); wg2_d = din("wg2", [F, F])
    b0c_d = din("b0c", [P, 1]); bg1c_d = din("bg1c", [P, 1]); bg2c_d = din("bg2c", [P, 1])
    wh1_d = din("wh1", [F, NHID]); bh1c_d = din("bh1c", [P, 2])
    wh2_d = din("wh2", [NHID, NOUT]); bh2rep_d = din("bh2rep", [P, NOUT])
    batA_d = din("batA", [P, NBLK]); batB_d = din("batB", [P, NBLK])
    invcA_d = din("invcA", [P, 1]); invcB_d = din("invcB", [P, 1])
    out_d = nc.declare_dram_parameter("out", [G, NOUT], F32, isOutput=True)

    slice0 = nc.dram_tensor("slice0", [NPC, F], F32)
    slice1 = nc.dram_tensor("slice1", [NPC, F], F32)
    slice2 = nc.dram_tensor("slice2", [NPC, F], F32)
    tab1 = nc.dram_tensor("tab1", [NPAD, F], F32, addr_space="Shared")
    tab2 = nc.dram_tensor("tab2", [NPAD, F], F32, addr_space="Shared")
    tab3 = nc.dram_tensor("tab3", [NPAD, F], F32, addr_space="Shared")
    pool_in = nc.dram_tensor("pool_in", [G, F], F32)
    pool_out = nc.dram_tensor("pool_out", [G, F], F32, addr_space="Shared")
    groups = [list(range(NCORES))]

    with tile.TileContext(nc) as tc:
        with (
            tc.tile_pool(name="const", bufs=1) as constp,
            tc.tile_pool(name="meta", bufs=1) as metap,
            tc.tile_pool(name="msg", bufs=6) as msgp,
            tc.tile_pool(name="sel", bufs=4) as selp,
            tc.tile_pool(name="work", bufs=6) as workp,
            tc.tile_pool(name="pagg", bufs=2, space="PSUM") as pagg,
            tc.tile_pool(name="phT", bufs=2, space="PSUM") as phT,
            tc.tile_pool(name="ptr", bufs=1, space="PSUM") as ptr,
            tc.tile_pool(name="ppool", bufs=1, space="PSUM") as ppool,
        ):
            # ---- constants / metadata to SBUF
            ident = constp.tile([P, P], F32)
            make_identity(nc, ident[:])
            iota = constp.tile([P, P], F32)
            nc.sync.dma_start(out=iota[:], in_=iota_d[:])

            def load(t_shape, dram, dt=F32, pool=metap):
                nm = f"sb_{dram.name}"
                t = pool.tile(list(t_shape), dt, name=nm, tag=nm)
                nc.sync.dma_start(out=t[:], in_=dram[:])
                return t

            idxlo = load([P, NGLO * NWG], idxlo_d, I16)
            idxhi = load([P, NGHI * NWG], idxhi_d, I16)
            dofflo = load([P, NBLK * NTLO], dofflo_d)
            doffhi = load([P, NBLK * NTHI], doffhi_d)
            vallo = load([P, NBLK * NTLO], vallo_d)
            valhi = load([P, NBLK * NTHI], valhi_d)
            w0 = load([F, F], w0_d, pool=constp)
            wg1 = load([F, F], wg1_d, pool=constp)
            wg2 = load([F, F], wg2_d, pool=constp)
            b0c = load([P, 1], b0c_d, pool=constp)
            bg1c = load([P, 1], bg1c_d, pool=constp)
            bg2c = load([P, 1], bg2c_d, pool=constp)
            wh1 = load([F, NHID], wh1_d, pool=constp)
            bh1c = load([P, 2], bh1c_d, pool=constp)
            wh2 = constp.tile([P, (NHID // P) * NOUT], F32)
            for h in range(NHID // P):
                nc.sync.dma_start(out=wh2[:, h * NOUT:(h + 1) * NOUT],
                                  in_=wh2_d[h * P:(h + 1) * P, :])
            bh2rep = load([P, NOUT], bh2rep_d, pool=constp)
            batA = load([P, NBLK], batA_d, pool=constp)
            batB = load([P, NBLK], batB_d, pool=constp)
            invcA = load([P, 1], invcA_d, pool=constp)
            invcB = load([P, 1], invcB_d, pool=constp)

            # stage xs -> slice0 -> tab1 (collectives need internal tensors)
            for b in range(NBLK):
                t = workp.tile([P, F], F32)
                nc.sync.dma_start(out=t[:], in_=xs_d[b * P:(b + 1) * P, :])
                nc.sync.dma_start(out=slice0[b * P:(b + 1) * P, :], in_=t[:])
            nc.gpsimd.collective_compute(
                "AllGather", mybir.AluOpType.bypass, replica_groups=groups,
                ins=[slice0[:]], outs=[tab1[:]])

            pool_ps = {}

            def emit_layer(L, tab, W_sb, bias_col, use_val, out_slice):
                stream_info = [
                    ("lo", NTLO, idxlo, dofflo, vallo, tab[0:NHALF, :]),
                    ("hi", NTHI, idxhi, doffhi, valhi, tab[NHALF:NPAD, :]),
                ]
                gbufs = {"lo": {}, "hi": {}}

                def get_gather(sname, g, idx_sb, tab_ap):
                    d = gbufs[sname]
                    if g not in d:
                        buf = msgp.tile([P, GATHER_ROWS], F32)
                        nc.gpsimd.dma_gather(
                            out_ap=buf[:].rearrange("p (c f) -> p c f", f=F),
                            in_ap=tab_ap,
                            idxs_ap=idx_sb[:, g * NWG:(g + 1) * NWG],
                            num_idxs=GATHER_ROWS, num_idxs_reg=GATHER_ROWS,
                            elem_size=F, single_packet=True,
                            queue_num=(L * NBLK + g) % NQ)
                        d[g] = buf
                    return d[g]

                for b in range(NBLK):
                    agg_ps = pagg.tile([P, F], F32, space="PSUM", tag="agg")
                    first = True
                    for sname, NT, idx_sb, doff_sb, val_sb, tab_ap in stream_info:
                        for tt in range(NT):
                            t = b * NT + tt
                            g, ch = divmod(t, CHUNKS)
                            buf = get_gather(sname, g, idx_sb, tab_ap)
                            sel = selp.tile([P, P], F32)
                            col = slice(b * NT + tt, b * NT + tt + 1)
                            if use_val:
                                nc.vector.tensor_scalar(
                                    out=sel[:], in0=iota[:],
                                    scalar1=doff_sb[:, col],
                                    scalar2=val_sb[:, col],
                                    op0=mybir.AluOpType.is_equal,
                                    op1=mybir.AluOpType.mult)
                            else:
                                nc.vector.tensor_scalar(
                                    out=sel[:], in0=iota[:],
                                    scalar1=doff_sb[:, col], scalar2=None,
                                    op0=mybir.AluOpType.is_equal)
                            last = (sname == "hi" and tt == NTHI - 1)
                            nc.tensor.matmul(
                                out=agg_ps[:],
                                lhsT=buf[:, ch * F:(ch + 1) * F],
                                rhs=sel[:], start=first, stop=last)
                            first = False
                    aggT = workp.tile([P, F], F32)
                    nc.vector.tensor_copy(out=aggT[:], in_=agg_ps[:])
                    hT_ps = phT.tile([P, F], F32, space="PSUM", tag="hT")
                    nc.tensor.matmul(out=hT_ps[:], lhsT=W_sb[:], rhs=aggT[:],
                                     start=True, stop=True)
                    hT = workp.tile([P, F], F32)
                    nc.scalar.activation(out=hT[:], in_=hT_ps[:],
                                         func=mybir.ActivationFunctionType.Relu,
                                         bias=bias_col[:, 0:1])
                    h_ps = ptr.tile([P, F], F32, space="PSUM", tag="tr")
                    nc.tensor.transpose(out=h_ps[:], in_=hT[:], identity=ident[:])
                    h_sb = workp.tile([P, F], F32)
                    nc.vector.tensor_copy(out=h_sb[:], in_=h_ps[:])
                    if out_slice is not None:
                        nc.sync.dma_start(out=out_slice[b * P:(b + 1) * P, :],
                                          in_=h_sb[:])
                    else:
                        for half, bat in (("A", batA), ("B", batB)):
                            if half not in pool_ps:
                                pool_ps[half] = ppool.tile(
                                    [P, F], F32, space="PSUM",
                                    tag=f"pool{half}", name=f"pool{half}")
                            selp_t = selp.tile([P, P], F32)
                            nc.vector.tensor_scalar(
                                out=selp_t[:], in0=iota[:],
                                scalar1=bat[:, b:b + 1], scalar2=None,
                                op0=mybir.AluOpType.is_equal)
                            nc.tensor.matmul(
                                out=pool_ps[half][:], lhsT=selp_t[:], rhs=h_sb[:],
                                start=(b == 0), stop=(b == NBLK - 1))

            emit_layer(0, tab1, w0, b0c, True, slice1)
            nc.gpsimd.collective_compute(
                "AllGather", mybir.AluOpType.bypass, replica_groups=groups,
                ins=[slice1[:]], outs=[tab2[:]])
            emit_layer(1, tab2, wg1, bg1c, False, slice2)
            nc.gpsimd.collective_compute(
                "AllGather", mybir.AluOpType.bypass, replica_groups=groups,
                ins=[slice2[:]], outs=[tab3[:]])
            emit_layer(2, tab3, wg2, bg2c, False, None)

            # ---- pooling: partial means -> AllReduce
            for half, invc in (("A", invcA), ("B", invcB)):
                m_sb = workp.tile([P, F], F32, tag=f"m{half}")
                nc.vector.tensor_scalar(
                    out=m_sb[:], in0=pool_ps[half][:], scalar1=invc[:, 0:1],
                    scalar2=None, op0=mybir.AluOpType.mult)
                base = 0 if half == "A" else P
                nc.sync.dma_start(out=pool_in[base:base + P, :], in_=m_sb[:])
            nc.gpsimd.collective_compute(
                "AllReduce", mybir.AluOpType.add, replica_groups=groups,
                ins=[pool_in[:]], outs=[pool_out[:]])

            # ---- head (redundant on every core)
            g1T = {}
            for hi, half in enumerate(("A", "B")):
                m_sb = workp.tile([P, F], F32, tag=f"mf{half}")
                nc.sync.dma_start(out=m_sb[:], in_=pool_out[hi * P:(hi + 1) * P, :])
                mT_ps = phT.tile([P, F], F32, space="PSUM", tag="hT")
                nc.tensor.transpose(out=mT_ps[:], in_=m_sb[:], identity=ident[:])
                mT = workp.tile([P, F], F32, tag=f"mT{half}")
                nc.vector.tensor_copy(out=mT[:], in_=mT_ps[:])
                for h in range(NHID // P):
                    g_ps = pagg.tile([P, P], F32, space="PSUM", tag="agg")
                    nc.tensor.matmul(out=g_ps[:], lhsT=wh1[:, h * P:(h + 1) * P],
                                     rhs=mT[:], start=True, stop=True)
                    gt = workp.tile([P, P], F32, tag=f"g1T{half}{h}")
                    nc.scalar.activation(out=gt[:], in_=g_ps[:],
                                         func=mybir.ActivationFunctionType.Relu,
                                         bias=bh1c[:, h:h + 1])
                    g1T[(half, h)] = gt
            for hi, half in enumerate(("A", "B")):
                o_ps = pagg.tile([P, NOUT], F32, space="PSUM", tag="agg")
                for h in range(NHID // P):
                    nc.tensor.matmul(out=o_ps[:], lhsT=g1T[(half, h)][:],
                                     rhs=wh2[:, h * NOUT:(h + 1) * NOUT],
                                     start=(h == 0), stop=(h == NHID // P - 1))
                o_sb = workp.tile([P, NOUT], F32, tag=f"o{half}")
                nc.vector.tensor_add(out=o_sb[:], in0=o_ps[:], in1=bh2rep[:])
                nc.sync.dma_start(out=out_d[hi * P:(hi + 1) * P, :], in_=o_sb[:])

    nc.compile()
    return nc


_CACHE = {}


def run(cfg, inputs):
    in_maps, meta = preprocess(cfg, **inputs)
    key = (cfg.N, meta["NTLO"], meta["NTHI"], meta["NGLO"], meta["NGHI"])
    if key not in _CACHE:
        _CACHE[key] = build_program(cfg, meta)
    nc = _CACHE[key]
    res = run_bass_kernel_spmd(nc, in_maps, core_ids=list(range(NCORES)))
    return res.results[0]["out"].astype(np.float32)


def kernel(**inputs):
    return run(FULL, inputs)


# revision 6
# speedup vs baseline: 1.0066x; 1.0066x over previous
"""GCN+GIN graph encoder on 8 Trainium2 NeuronCores (Bass/Tile).

Math (reference):
  GCNConv:  h = relu(segsum_dst(norm_e * (x@W0)[src]) + b0),
            norm_e = dinv[src]*dinv[dst] over edges+self-loops,
            dinv = rsqrt(deg incl self-loop)
  GIN x2:   h = relu((h + segsum_dst(h[src])) @ Wg + bg)
  pool:     m = segment_mean(h, batch) -> relu(m@Wh1+bh1)@Wh2+bh2

Distribution: nodes (and their in-edges) sharded contiguously over 8 cores.
Per layer each core aggregates messages for its own dst nodes by gathering
rows of a replicated node-feature table (dma_gather, 1024-row packed ops on
4 SWDGE queues), reducing edge tiles with one-hot selection matrices on the
TensorEngine, applying the layer linear transform W-stationary in feat-major,
then transposing back to node-major.  Tables are re-replicated between layers
with an AllGather; pooled partial means are combined with an AllReduce and
the small MLP head is computed redundantly on every core.

Aggregation identity per dst block b (128 dst nodes):
  aggT[f, d] = sum_e msg[e, f] * sel[e, d],  sel[e, d] = (doff[e] == d) * val[e]
computed as matmul(lhsT=msg_tile[128e, 128f], rhs=sel[128e, 128d]) accumulated
in PSUM over the block's edge tiles.  GCN folds dinv[src] into the table rows
(host-prescaled x) and dinv[dst] into val; GIN uses val=1 and a self-loop edge
supplies the "+h" term.  Pad edge slots carry doff=-1 -> zero contribution.
"""
import sys
import os

sys.path.insert(0, '/opt/trn_rl_repo')

import numpy as np

import concourse.bass as bass
import concourse.bacc as bacc
import concourse.mybir as mybir
import concourse.tile as tile
from concourse.bass_utils import run_bass_kernel_spmd
from concourse.masks import make_identity

F32 = mybir.dt.float32
I16 = mybir.dt.int16
P = 128
NCORES = 8
GATHER_ROWS = 1024          # rows per dma_gather (single_packet limit)
NQ = 4                      # SWDGE queues


class Cfg:
    def __init__(self, N, E, G, F, NHID, NOUT, NPN):
        self.N = N            # real nodes
        self.E = E            # edges (no self loops)
        self.G = G            # graphs
        self.F = F            # feature/hidden width (128)
        self.NHID = NHID
        self.NOUT = NOUT
        self.NPN = NPN        # real nodes per core
        assert NPN * NCORES >= N > NPN * (NCORES - 1)
        self.NPC = ((NPN + P - 1) // P) * P   # padded nodes per core
        self.NBLK = self.NPC // P
        self.NPAD = self.NPC * NCORES
        self.NHALF = self.NPAD // 2
        assert self.NHALF < 32768
        assert G == 2 * P


FULL = Cfg(N=50000, E=800000, G=256, F=128, NHID=256, NOUT=128, NPN=6250)


# ---------------------------------------------------------------- host prep
def preprocess(cfg, x, edge_index, batch, W0, b0, Wg1, bg1, Wg2, bg2,
               Wh1, bh1, Wh2, bh2):
    N, G, F = cfg.N, cfg.G, cfg.F
    NPN, NPC, NBLK, NPAD, NHALF = cfg.NPN, cfg.NPC, cfg.NBLK, cfg.NPAD, cfg.NHALF

    src = np.asarray(edge_index[0], dtype=np.int64)
    dst = np.asarray(edge_index[1], dtype=np.int64)
    batch = np.asarray(batch, dtype=np.int64)
    loop = np.arange(N, dtype=np.int64)
    s_all = np.concatenate([src, loop])
    d_all = np.concatenate([dst, loop])

    deg = np.bincount(d_all, minlength=N).astype(np.float64)
    dinv = (1.0 / np.sqrt(np.maximum(deg, 1.0))).astype(np.float32)

    def tabidx(n):
        c = n // NPN
        return c * NPC + (n - c * NPN)

    sidx = tabidx(s_all).astype(np.int64)
    c_e = d_all // NPN
    loc = d_all - c_e * NPN
    b_e = loc // P
    off_e = loc % P
    gblk = c_e * NBLK + b_e                      # global dst block id
    val_e = dinv[d_all].astype(np.float32)      # GCN dst scaling

    NGB = NCORES * NBLK
    streams = {}
    for name, mask in (("lo", sidx < NHALF), ("hi", sidx >= NHALF)):
        sg = gblk[mask]
        si = sidx[mask] - (0 if name == "lo" else NHALF)
        sof = off_e[mask]
        sva = val_e[mask]
        order = np.argsort(sg, kind="stable")
        sg, si, sof, sva = sg[order], si[order], sof[order], sva[order]
        cnt = np.bincount(sg, minlength=NGB)
        NT = int(np.ceil(cnt.max() / P)) if len(sg) else 1
        NT = max(NT, 1)
        rows_blk = NT * P
        starts = np.zeros(NGB, dtype=np.int64)
        starts[1:] = np.cumsum(cnt)[:-1]
        rank = np.arange(len(sg)) - np.repeat(starts, cnt)
        pos = sg * rows_blk + rank               # row in padded stream
        tot = NGB * rows_blk
        idx_arr = np.zeros(tot, dtype=np.int32)
        doff_arr = np.full(tot, -1.0, dtype=np.float32)
        val_arr = np.zeros(tot, dtype=np.float32)
        idx_arr[pos] = si
        doff_arr[pos] = sof
        val_arr[pos] = sva
        # per-core views
        rows_core = NBLK * rows_blk
        idx_arr = idx_arr.reshape(NCORES, rows_core)
        doff_arr = doff_arr.reshape(NCORES, rows_core)
        val_arr = val_arr.reshape(NCORES, rows_core)
        NG = (rows_core + GATHER_ROWS - 1) // GATHER_ROWS
        rows_g = NG * GATHER_ROWS
        pad = rows_g - rows_core
        if pad:
            idx_arr = np.pad(idx_arr, ((0, 0), (0, pad)))
        # wrap int16 for dma_gather: element i -> partition i%16, col i//16
        NWG = GATHER_ROWS // 16
        wrapped = idx_arr.reshape(NCORES, NG, NWG, 16).transpose(0, 3, 1, 2)
        wrapped = wrapped.reshape(NCORES, 16, NG * NWG).astype(np.int16)
        wrapped = np.tile(wrapped, (1, 8, 1))    # [NCORES, 128, NG*NWG]
        # doff/val tile-major: [T=NBLK*NT, 128] -> [128, T]
        T = NBLK * NT
        doff2 = doff_arr.reshape(NCORES, T, P).transpose(0, 2, 1).copy()
        val2 = val_arr.reshape(NCORES, T, P).transpose(0, 2, 1).copy()
        streams[name] = dict(NT=NT, NG=NG, idx=wrapped, doff=doff2, val=val2)

    # per-core node-feature slice, pre-scaled by dinv (GCN source scaling)
    xs = np.zeros((NCORES, NPC, F), dtype=np.float32)
    x = np.asarray(x, dtype=np.float32)
    for c in range(NCORES):
        lo_n = c * NPN
        hi_n = min(N, (c + 1) * NPN)
        n = hi_n - lo_n
        xs[c, :n] = x[lo_n:hi_n] * dinv[lo_n:hi_n, None]

    # pooling metadata
    cnt_g = np.bincount(batch, minlength=G).astype(np.float32)
    invc = (1.0 / np.maximum(cnt_g, 1.0)).astype(np.float32)
    batA = np.full((NCORES, P, NBLK), -1.0, dtype=np.float32)
    batB = np.full((NCORES, P, NBLK), -1000.0, dtype=np.float32)
    for c in range(NCORES):
        lo_n = c * NPN
        hi_n = min(N, (c + 1) * NPN)
        n = hi_n - lo_n
        bb = batch[lo_n:hi_n].astype(np.float32)
        colmaj = np.full(NPC, -1.0, dtype=np.float32)
        colmaj[:n] = bb
        batA[c] = colmaj.reshape(NBLK, P).T
        batB[c] = batA[c] - 128.0
        batA[c][batA[c] < 0] = -1.0

    iota = np.broadcast_to(np.arange(P, dtype=np.float32), (P, P)).copy()

    common = dict(
        iota=iota,
        w0=np.asarray(W0, np.float32), wg1=np.asarray(Wg1, np.float32),
        wg2=np.asarray(Wg2, np.float32),
        b0c=np.asarray(b0, np.float32).reshape(P, 1).copy(),
        bg1c=np.asarray(bg1, np.float32).reshape(P, 1).copy(),
        bg2c=np.asarray(bg2, np.float32).reshape(P, 1).copy(),
        wh1=np.asarray(Wh1, np.float32),
        bh1c=np.asarray(bh1, np.float32).reshape(2, P).T.copy(),  # [128,2]
        wh2=np.asarray(Wh2, np.float32),
        bh2rep=np.broadcast_to(np.asarray(bh2, np.float32), (P, cfg.NOUT)).copy(),
        invcA=invc[:P].reshape(P, 1).copy(),
        invcB=invc[P:].reshape(P, 1).copy(),
    )
    in_maps = []
    for c in range(NCORES):
        m = dict(common)
        m.update(
            xs=xs[c],
            idxlo=streams["lo"]["idx"][c], idxhi=streams["hi"]["idx"][c],
            dofflo=streams["lo"]["doff"][c], doffhi=streams["hi"]["doff"][c],
            vallo=streams["lo"]["val"][c], valhi=streams["hi"]["val"][c],
            batA=batA[c], batB=batB[c],
        )
        in_maps.append(m)
    meta = dict(NTLO=streams["lo"]["NT"], NGLO=streams["lo"]["NG"],
                NTHI=streams["hi"]["NT"], NGHI=streams["hi"]["NG"])
    return in_maps, meta


# ---------------------------------------------------------------- program
def build_program(cfg, meta):
    NPC, NBLK, NPAD, NHALF = cfg.NPC, cfg.NBLK, cfg.NPAD, cfg.NHALF
    F, NHID, NOUT, G = cfg.F, cfg.NHID, cfg.NOUT, cfg.G
    NTLO, NGLO = meta["NTLO"], meta["NGLO"]
    NTHI, NGHI = meta["NTHI"], meta["NGHI"]
    NWG = GATHER_ROWS // 16
    CHUNKS = GATHER_ROWS // P     # 8 message tiles per gather

    nc = bacc.Bacc(None, target_bir_lowering=False, debug=True,
                   num_devices=NCORES, num_swdge_queues=NQ)

    def din(name, shape, dt=F32):
        return nc.declare_dram_parameter(name, list(shape), dt, isOutput=False)

    xs_d = din("xs", [NPC, F])
    idxlo_d = din("idxlo", [P, NGLO * NWG], I16)
    idxhi_d = din("idxhi", [P, NGHI * NWG], I16)
    dofflo_d = din("dofflo", [P, NBLK * NTLO])
    doffhi_d = din("doffhi", [P, NBLK * NTHI])
    vallo_d = din("vallo", [P, NBLK * NTLO])
    valhi_d = din("valhi", [P, NBLK * NTHI])
    iota_d = din("iota", [P, P])
    w0_d = din("w0", [F, F]); wg1_d = din("wg1", [F, F]); wg2_d = din("wg2", [F, F])
    b0c_d = din("b0c", [P, 1]); bg1c_d = din("bg1c", [P, 1]); bg2c_d = din("bg2c", [P, 1])
    wh1_d = din("wh1", [F, NHID]); bh1c_d = din("bh1c", [P, 2])
    wh2_d = din("wh2", [NHID, NOUT]); bh2rep_d = din("bh2rep", [P, NOUT])
    batA_d = din("batA", [P, NBLK]); batB_d = din("batB", [P, NBLK])
    invcA_d = din("invcA", [P, 1]); invcB_d = din("invcB", [P, 1])
    out_d = nc.declare_dram_parameter("out", [G, NOUT], F32, isOutput=True)

    slice0 = nc.dram_tensor("slice0", [NPC, F], F32)
    slice1 = nc.dram_tensor("slice1", [NPC, F], F32)
    slice2 = nc.dram_tensor("slice2", [NPC, F], F32)
    tab1 = nc.dram_tensor("tab1", [NPAD, F], F32)
    tab2 = nc.dram_tensor("tab2", [NPAD, F], F32)
    tab3 = nc.dram_tensor("tab3", [NPAD, F], F32)
    pool_in = nc.dram_tensor("pool_in", [G, F], F32)
    pool_out = nc.dram_tensor("pool_out", [G, F], F32, addr_space="Shared")
    groups = [list(range(NCORES))]

    with tile.TileContext(nc) as tc:
        with (
            tc.tile_pool(name="const", bufs=1) as constp,
            tc.tile_pool(name="meta", bufs=1) as metap,
            tc.tile_pool(name="msg", bufs=6) as msgp,
            tc.tile_pool(name="sel", bufs=4) as selp,
            tc.tile_pool(name="work", bufs=6) as workp,
            tc.tile_pool(name="pagg", bufs=2, space="PSUM") as pagg,
            tc.tile_pool(name="phT", bufs=2, space="PSUM") as phT,
            tc.tile_pool(name="ptr", bufs=1, space="PSUM") as ptr,
            tc.tile_pool(name="ppool", bufs=1, space="PSUM") as ppool,
        ):
            # ---- constants / metadata to SBUF
            ident = constp.tile([P, P], F32)
            make_identity(nc, ident[:])
            iota = constp.tile([P, P], F32)
            nc.sync.dma_start(out=iota[:], in_=iota_d[:])

            def load(t_shape, dram, dt=F32, pool=metap):
                nm = f"sb_{dram.name}"
                t = pool.tile(list(t_shape), dt, name=nm, tag=nm)
                nc.sync.dma_start(out=t[:], in_=dram[:])
                return t

            idxlo = load([P, NGLO * NWG], idxlo_d, I16)
            idxhi = load([P, NGHI * NWG], idxhi_d, I16)
            dofflo = load([P, NBLK * NTLO], dofflo_d)
            doffhi = load([P, NBLK * NTHI], doffhi_d)
            vallo = load([P, NBLK * NTLO], vallo_d)
            valhi = load([P, NBLK * NTHI], valhi_d)
            w0 = load([F, F], w0_d, pool=constp)
            wg1 = load([F, F], wg1_d, pool=constp)
            wg2 = load([F, F], wg2_d, pool=constp)
            b0c = load([P, 1], b0c_d, pool=constp)
            bg1c = load([P, 1], bg1c_d, pool=constp)
            bg2c = load([P, 1], bg2c_d, pool=constp)
            wh1 = load([F, NHID], wh1_d, pool=constp)
            bh1c = load([P, 2], bh1c_d, pool=constp)
            wh2 = constp.tile([P, (NHID // P) * NOUT], F32)
            for h in range(NHID // P):
                nc.sync.dma_start(out=wh2[:, h * NOUT:(h + 1) * NOUT],
                                  in_=wh2_d[h * P:(h + 1) * P, :])
            bh2rep = load([P, NOUT], bh2rep_d, pool=constp)
            batA = load([P, NBLK], batA_d, pool=constp)
            batB = load([P, NBLK], batB_d, pool=constp)
            invcA = load([P, 1], invcA_d, pool=constp)
            invcB = load([P, 1], invcB_d, pool=constp)

            # stage xs -> slice0 -> tab1 (collectives need internal tensors)
            for b in range(NBLK):
                t = workp.tile([P, F], F32)
                nc.sync.dma_start(out=t[:], in_=xs_d[b * P:(b + 1) * P, :])
                nc.sync.dma_start(out=slice0[b * P:(b + 1) * P, :], in_=t[:])
            nc.gpsimd.collective_compute(
                "AllGather", mybir.AluOpType.bypass, replica_groups=groups,
                ins=[slice0[:]], outs=[tab1[:]])

            pool_ps = {}

            def emit_layer(L, tab, W_sb, bias_col, use_val, out_slice):
                stream_info = [
                    ("lo", NTLO, idxlo, dofflo, vallo, tab[0:NHALF, :]),
                    ("hi", NTHI, idxhi, doffhi, valhi, tab[NHALF:NPAD, :]),
                ]
                gbufs = {"lo": {}, "hi": {}}

                def get_gather(sname, g, idx_sb, tab_ap):
                    d = gbufs[sname]
                    if g not in d:
                        buf = msgp.tile([P, GATHER_ROWS], F32)
                        nc.gpsimd.dma_gather(
                            out_ap=buf[:].rearrange("p (c f) -> p c f", f=F),
                            in_ap=tab_ap,
                            idxs_ap=idx_sb[:, g * NWG:(g + 1) * NWG],
                            num_idxs=GATHER_ROWS, num_idxs_reg=GATHER_ROWS,
                            elem_size=F, single_packet=True,
                            queue_num=(L * NBLK + g) % NQ)
                        d[g] = buf
                    return d[g]

                for b in range(NBLK):
                    agg_ps = pagg.tile([P, F], F32, space="PSUM", tag="agg")
                    first = True
                    for sname, NT, idx_sb, doff_sb, val_sb, tab_ap in stream_info:
                        for tt in range(NT):
                            t = b * NT + tt
                            g, ch = divmod(t, CHUNKS)
                            buf = get_gather(sname, g, idx_sb, tab_ap)
                            sel = selp.tile([P, P], F32)
                            col = slice(b * NT + tt, b * NT + tt + 1)
                            if use_val:
                                nc.vector.tensor_scalar(
                                    out=sel[:], in0=iota[:],
                                    scalar1=doff_sb[:, col],
                                    scalar2=val_sb[:, col],
                                    op0=mybir.AluOpType.is_equal,
                                    op1=mybir.AluOpType.mult)
                            else:
                                nc.vector.tensor_scalar(
                                    out=sel[:], in0=iota[:],
                                    scalar1=doff_sb[:, col], scalar2=None,
                                    op0=mybir.AluOpType.is_equal)
                            last = (sname == "hi" and tt == NTHI - 1)
                            nc.tensor.matmul(
                                out=agg_ps[:],
                                lhsT=buf[:, ch * F:(ch + 1) * F],
                                rhs=sel[:], start=first, stop=last)
                            first = False
                    aggT = workp.tile([P, F], F32)
                    nc.vector.tensor_copy(out=aggT[:], in_=agg_ps[:])
                    hT_ps = phT.tile([P, F], F32, space="PSUM", tag="hT")
                    nc.tensor.matmul(out=hT_ps[:], lhsT=W_sb[:], rhs=aggT[:],
                                     start=True, stop=True)
                    hT = workp.tile([P, F], F32)
                    nc.scalar.activation(out=hT[:], in_=hT_ps[:],
                                         func=mybir.ActivationFunctionType.Relu,
                                         bias=bias_col[:, 0:1])
                    h_ps = ptr.tile([P, F], F32, space="PSUM", tag="tr")
                    nc.tensor.transpose(out=h_ps[:], in_=hT[:], identity=ident[:])
                    h_sb = workp.tile([P, F], F32)
                    nc.vector.tensor_copy(out=h_sb[:], in_=h_ps[:])
                    if out_slice is not None:
                        nc.sync.dma_start(out=out_slice[b * P:(b + 1) * P, :],
                                          in_=h_sb[:])
                    else:
                        for half, bat in (("A", batA), ("B", batB)):
                            if half not in pool_ps:
                                pool_ps[half] = ppool.tile(
                                    [P, F], F32, space="PSUM",
                                    tag=f"pool{half}", name=f"pool{half}")
                            selp_t = selp.tile([P, P], F32)
                            nc.vector.tensor_scalar(
                                out=selp_t[:], in0=iota[:],
                                scalar1=bat[:, b:b + 1], scalar2=None,
                                op0=mybir.AluOpType.is_equal)
                            nc.tensor.matmul(
                                out=pool_ps[half][:], lhsT=selp_t[:], rhs=h_sb[:],
                                start=(b == 0), stop=(b == NBLK - 1))

            emit_layer(0, tab1, w0, b0c, True, slice1)
            nc.gpsimd.collective_compute(
                "AllGather", mybir.AluOpType.bypass, replica_groups=groups,
                ins=[slice1[:]], outs=[tab2[:]])
            emit_layer(1, tab2, wg1, bg1c, False, slice2)
            nc.gpsimd.collective_compute(
                "AllGather", mybir.AluOpType.bypass, replica_groups=groups,
                ins=[slice2[:]], outs=[tab3[:]])
            emit_layer(2, tab3, wg2, bg2c, False, None)

            # ---- pooling: partial means -> AllReduce
            for half, invc in (("A", invcA), ("B", invcB)):
                m_sb = workp.tile([P, F], F32, tag=f"m{half}")
                nc.vector.tensor_scalar(
                    out=m_sb[:], in0=pool_ps[half][:], scalar1=invc[:, 0:1],
                    scalar2=None, op0=mybir.AluOpType.mult)
                base = 0 if half == "A" else P
                nc.sync.dma_start(out=pool_in[base:base + P, :], in_=m_sb[:])
            nc.gpsimd.collective_compute(
                "AllReduce", mybir.AluOpType.add, replica_groups=groups,
                ins=[pool_in[:]], outs=[pool_out[:]])

            # ---- head (redundant on every core)
            g1T = {}
            for hi, half in enumerate(("A", "B")):
                m_sb = workp.tile([P, F], F32, tag=f"mf{half}")
                nc.sync.dma_start(out=m_sb[:], in_=pool_out[hi * P:(hi + 1) * P, :])
                mT_ps = phT.tile([P, F], F32, space="PSUM", tag="hT")
                nc.tensor.transpose(out=mT_ps[:], in_=m_sb[:], identity=ident[:])
                mT = workp.tile([P, F], F32, tag=f"mT{half}")
                nc.vector.tensor_copy(out=mT[:], in_=mT_ps[:])
                for h in range(NHID // P):
                    g_ps = pagg.tile([P, P], F32, space="PSUM", tag="agg")
                    nc.tensor.matmul(out=g_ps[:], lhsT=wh1[:, h * P:(h + 1) * P],
                                     rhs=mT[:], start=True, stop=True)
                    gt = workp.tile([P, P], F32, tag=f"g1T{half}{h}")
                    nc.scalar.activation(out=gt[:], in_=g_ps[:],
                                         func=mybir.ActivationFunctionType.Relu,
                                         bias=bh1c[:, h:h + 1])
                    g1T[(half, h)] = gt
            for hi, half in enumerate(("A", "B")):
                o_ps = pagg.tile([P, NOUT], F32, space="PSUM", tag="agg")
                for h in range(NHID // P):
                    nc.tensor.matmul(out=o_ps[:], lhsT=g1T[(half, h)][:],
                                     rhs=wh2[:, h * NOUT:(h + 1) * NOUT],
                                     start=(h == 0), stop=(h == NHID // P - 1))
                o_sb = workp.tile([P, NOUT], F32, tag=f"o{half}")
                nc.vector.tensor_add(out=o_sb[:], in0=o_ps[:], in1=bh2rep[:])
                nc.sync.dma_start(out=out_d[hi * P:(hi + 1) * P, :], in_=o_sb[:])

    nc.compile()
    return nc


_CACHE = {}


def run(cfg, inputs):
    in_maps, meta = preprocess(cfg, **inputs)
    key = (cfg.N, meta["NTLO"], meta["NTHI"], meta["NGLO"], meta["NGHI"])
    if key not in _CACHE:
        _CACHE[key] = build_program(cfg, meta)
    nc = _CACHE[key]
    res = run_bass_kernel_spmd(nc, in_maps, core_ids=list(range(NCORES)))
    return res.results[0]["out"].astype(np.float32)


def kernel(**inputs):
    return run(FULL, inputs)
